# revision 6
# baseline (speedup 1.0000x reference)
"""Trainium2 Bass kernel for nn_AttentionCore64: softmax(Q@K^T)@V (raw exp,
no scaling), B=2 H=16 S=2048 D=64, f32 in/out. B*H sharded over 8 cores.

Final: v1 steady-state structure (per-2-chunk fin on SP, 11/5 ACT/DVE
exp split, PV delays 2/4), with three boundary-stall fixes on top:
- the chunk's FIRST PV (the only one gated on the previous chunk's fin
  copies via out_ps WAR) is delayed to t+5, and the PV queue is a strict
  t-order FIFO (start_tensor_calc=True must stay first per accumulator),
  so the in-order PE queue runs ~5 tiles of independent QK/exp ahead at
  every chunk boundary;
- BOTH fin staging copies run on DVE, keeping the ACT exp stream gapless
  across the boundary (exp(c,0) issues right after exp(c-1,15));
- bootstrap g0 converts on DVE (no GPSIMD Q7 launch in the first-QK
  chain). Plus:
- act-table warmup exp at t=0 (overlaps the 1.3us LoadActFuncSet).
- 512B DMA descriptors: interleaved row-pair HBM layout for q/k/v/out
  ("(a p b) d -> p a b d", row s = 256a+2p+b) -> 2x DMA throughput.
  Key order inside each kT tile is permuted, but vx rows (from v in the
  same layout) match; query order permutes score columns, and the store
  uses the same layout, so results are identical.
- Fast bootstrap: k g0-1 + q g0 loads and their transposes first (first
  QK at ~6us, was 13.9us); q g1-3 loads also in the bootstrap so the
  chunk-1 qT transpose has ~10us of lead (was the 1.5-4.8us stall).
- q/k fp32->bf16 converts on GPSIMD (was ACT/DVE): the convert chain no
  longer queues behind exp bursts.
- pair-1 setup kicked off at chunk 1 (was chunk 2): +13.6us of lead.
"""

import numpy as np
from contextlib import ExitStack

import concourse.tile as tile
import concourse.mybir as mybir
from concourse import bacc
from concourse.bass_utils import run_bass_kernel_spmd

B, H, S, D = 2, 16, 2048, 64
NCORES = 8
HPC = (B * H) // NCORES  # 4 heads per core

P = 128
CH = 512            # queries per chunk
NCH = S // CH       # 4 chunks per pair
NT = S // P         # 16 key tiles
NO = S // P         # 16 query o-tiles
DT = mybir.dt
AF = mybir.ActivationFunctionType
OP = mybir.AluOpType

PAIRS = [(0, 1), (2, 3)]
VW = 80             # padded PV weight cols: 64 v + 1 ones + 15 zeros

A16 = float(128.0 / np.log(2.0))
B16 = 16256.0 - 7.30  # calibrated: E[schraudolph/exp] = 1.0003

RLOAD = "(a p b) d -> p a b d"  # interleaved row-pair layout (512B desc)

DVE_T = frozenset((2, 5, 8, 11, 14))
SCALAR_T = frozenset(t for t in range(16) if t not in DVE_T)
PVD_SCALAR = 2
PVD_DVE = 4
PVD_T0 = 5


def build(reps=None):
    nc = bacc.Bacc("TRN2", target_bir_lowering=False, debug=False)
    q_ext = nc.dram_tensor("q", [HPC, S, D], DT.float32, kind="ExternalInput").ap()
    k_ext = nc.dram_tensor("k", [HPC, S, D], DT.float32, kind="ExternalInput").ap()
    v_ext = nc.dram_tensor("v", [HPC, S, D], DT.float32, kind="ExternalInput").ap()
    out_ext = nc.dram_tensor("out", [HPC, S, D], DT.float32, kind="ExternalOutput").ap()

    with tile.TileContext(nc) as tc, ExitStack() as ctx:
        if reps is not None:
            ctx.enter_context(tc.For_i(0, reps))
        sb = ctx.enter_context(tc.tile_pool(name="sb", bufs=2))
        pr = ctx.enter_context(tc.tile_pool(name="pr", bufs=6))
        ps_sc = ctx.enter_context(tc.tile_pool(name="ps_sc", bufs=3, space="PSUM"))
        ps_out = ctx.enter_context(tc.tile_pool(name="ps_out", bufs=1, space="PSUM"))

        # act-table warmup: first Exp triggers the 1.3us LoadActFuncSet.
        warm = sb.tile([1, 8], DT.float32, tag="warm", name="warm")
        warm_o = sb.tile([1, 8], DT.float32, tag="warm_o", name="warm_o")
        nc.gpsimd.memset(warm[:], 0.0)
        nc.scalar.activation(warm_o[:], warm[:], AF.Exp)

        # ---------------- per-pair state ----------------
        def alloc_pair(pi):
            st = {"pi": pi}
            for nm in ("q_nat", "k_nat", "v_nat"):
                st[nm] = sb.tile([P, 2, NO, D], DT.float32, tag=nm, name=nm)
            for nm in ("qbf", "kbf"):
                st[nm] = sb.tile([P, NO, 2, D], DT.bfloat16, tag=nm, name=nm)
            for nm in ("qT", "kT"):
                st[nm] = sb.tile([P, NO, P], DT.bfloat16, tag=nm, name=nm)
            st["vx"] = sb.tile([P, NT, 2, VW], DT.bfloat16, tag="vx", name="vx")
            return st

        def load_head(st, which, hi, g0, gn):
            """One 3D DMA: o-tile groups [g0, g0+gn) of one head (512B desc)."""
            hA, _ = PAIRS[st["pi"]]
            ext = {"q": q_ext, "k": k_ext, "v": v_ext}[which]
            dst = st[{"q": "q_nat", "k": "k_nat", "v": "v_nat"}[which]]
            sl = slice(g0 * 2, (g0 + gn) * 2)
            nc.sync.dma_start(
                dst[:, hi].rearrange("p (a b) d -> p a b d", b=2)[:, sl, :, :],
                ext[hA + hi].rearrange(RLOAD, p=P, b=2)[:, sl, :, :],
            )

        def convert_groups(st, which, g0, gn, eng=None):
            """fp32 head-major -> bf16 o-major for groups [g0, g0+gn), GPSIMD.
            Per-head copies: Q7 handles the 3D strided AP (4D crashes it)."""
            src = st[{"q": "q_nat", "k": "k_nat"}[which]]
            dst = st[{"q": "qbf", "k": "kbf"}[which]]
            sl = slice(g0 * 4, (g0 + gn) * 4)
            if eng == "dve":
                nc.vector.tensor_copy(
                    dst[:, sl, :, :],
                    src[:, :, sl, :].rearrange("p h o d -> p o h d"),
                )
                return
            for hi in (0, 1):
                nc.gpsimd.tensor_copy(dst[:, sl, hi, :], src[:, hi, sl, :])

        def vx_groups(st, g0, gn):
            """Build vx = [v | 1 | zeros] bf16 for groups [g0, g0+gn) (GPSIMD)."""
            sl = slice(g0 * 4, (g0 + gn) * 4)
            for hi in (0, 1):
                nc.gpsimd.tensor_copy(
                    st["vx"][:, sl, hi, :D], st["v_nat"][:, hi, sl, :]
                )
            nc.gpsimd.memset(st["vx"][:, sl, :, D], 1.0)
            nc.gpsimd.memset(st["vx"][:, sl, :, D + 1 :], 0.0)

        def tr_groups(st, which, g0, gn, eng=None):
            """One XBAR DMA transposing o-tiles [g0*4, (g0+gn)*4) blockwise."""
            src = st[{"q": "qbf", "k": "kbf"}[which]]
            dst = st[{"q": "qT", "k": "kT"}[which]]
            sl = slice(g0 * 4, (g0 + gn) * 4)
            e = nc.scalar if eng == "act" else nc.sync
            e.dma_start_transpose(dst[:, sl, :], src[:, sl, :, :])

        # ---------------- finalization (per 2-chunk group, on SP) ----------
        def emit_fin_copy(fin):
            out_ps = fin["out_ps"]
            cc = fin["c"] % 2
            if cc == 0:
                fin2 = {"pi": fin["pi"], "c0": fin["c"]}
                fin2["outT"] = sb.tile(
                    [VW, 2, 2, CH], DT.bfloat16, tag="outT", name="outT"
                )
            else:
                fin2 = fin["fin2"]
            outT = fin2["outT"]
            nc.vector.tensor_copy(outT[:, 0, cc, :], out_ps[:, :CH])
            nc.vector.tensor_copy(outT[:, 1, cc, :], out_ps[:, CH:])
            return fin2

        def emit_fin_tr(fin2):
            finT = sb.tile([P, 2, 8, VW], DT.bfloat16, tag="finT", name="finT")
            nc.sync.dma_start_transpose(finT[:], fin2["outT"][:])
            fin2["finT"] = finT

        def emit_fin_out(fin2):
            finT = fin2["finT"]  # [128, hi, (cc j), 80]
            hA, _ = PAIRS[fin2["pi"]]
            c0 = fin2["c0"]
            recip = sb.tile([P, 2, 8, 1], DT.float32, tag="recip", name="recip")
            nc.vector.reciprocal(recip[:], finT[:, :, :, D : D + 1])
            outn = sb.tile([P, 2, 8, D], DT.float32, tag="outn", name="outn")
            nc.vector.tensor_tensor(
                outn[:],
                finT[:, :, :, :D],
                recip[:].to_broadcast((P, 2, 8, D)),
                OP.mult,
            )
            for hi in (0, 1):
                nc.sync.dma_start(
                    out_ext[hA + hi].rearrange(RLOAD, p=P, b=2)[
                        :, c0 * 2 : c0 * 2 + 4, :, :
                    ],
                    outn[:, hi].rearrange("p (a b) d -> p a b d", b=2),
                )

        def emit_pv(st, out_ps, probs, t):
            nc.tensor.matmul(
                out_ps[:, :CH], st["vx"][:, t, 0, :], probs[:, :CH],
                start=(t == 0), stop=(t == NT - 1),
            )
            nc.tensor.matmul(
                out_ps[:, CH:], st["vx"][:, t, 1, :], probs[:, CH:],
                start=(t == 0), stop=(t == NT - 1),
            )

        # ---------------- pair-setup emission scheduling ----------------
        def p1_stream_ops(st):
            # plain loads strictly before the transposes that wait on their
            # converts (a waiting DMA head-blocks the SP sequencer); Pool
            # order = conv k -> vx -> conv q (by downstream deadline).
            return [
                lambda: load_head(st, "k", 0, 0, 4),
                lambda: load_head(st, "k", 1, 0, 4),
                lambda: load_head(st, "q", 0, 0, 4),
                lambda: load_head(st, "q", 1, 0, 4),
                lambda: load_head(st, "v", 0, 0, 4),
                lambda: load_head(st, "v", 1, 0, 4),
                lambda: convert_groups(st, "k", 0, 2),
                lambda: convert_groups(st, "k", 2, 2),
                lambda: tr_groups(st, "k", 0, 4),
                lambda: vx_groups(st, 0, 2),
                lambda: vx_groups(st, 2, 2),
                lambda: convert_groups(st, "q", 0, 2),
                lambda: convert_groups(st, "q", 2, 2),
                lambda: tr_groups(st, "q", 0, 4),
            ]

        # ------- bootstrap: entire pair-0 setup, minimal QK(0,0) chain first.
        # SP: the k g0-1 / q g0 chains feeding QK(0,0), then all remaining
        # loads, then the transposes that wait on Pool converts.
        st_cur = alloc_pair(0)
        load_head(st_cur, "k", 0, 0, 2)
        load_head(st_cur, "k", 1, 0, 2)
        load_head(st_cur, "q", 0, 0, 1)
        load_head(st_cur, "q", 1, 0, 1)
        convert_groups(st_cur, "k", 0, 1, eng="dve")
        convert_groups(st_cur, "q", 0, 1, eng="dve")
        tr_groups(st_cur, "k", 0, 1, eng="act")
        tr_groups(st_cur, "q", 0, 1, eng="act")
        load_head(st_cur, "v", 0, 0, 2)
        load_head(st_cur, "v", 1, 0, 2)
        load_head(st_cur, "k", 0, 2, 2)
        load_head(st_cur, "k", 1, 2, 2)
        load_head(st_cur, "q", 0, 1, 3)
        load_head(st_cur, "q", 1, 1, 3)
        load_head(st_cur, "v", 0, 2, 2)
        load_head(st_cur, "v", 1, 2, 2)
        convert_groups(st_cur, "k", 1, 1)
        tr_groups(st_cur, "k", 1, 1)
        vx_groups(st_cur, 0, 1)
        vx_groups(st_cur, 1, 1)
        convert_groups(st_cur, "k", 2, 2)
        tr_groups(st_cur, "k", 2, 2)
        vx_groups(st_cur, 2, 1)
        vx_groups(st_cur, 3, 1)
        convert_groups(st_cur, "q", 1, 3)
        tr_groups(st_cur, "q", 1, 3)
        pending_setup = []

        from collections import deque

        st_next = None
        # FIFO popped strictly in t-order: PV(t=0) carries start=True, and
        # with PVD_T0 > later delays a due-sorted flush would emit PV(1)
        # first and the start would zero its contribution.
        pending_pv = deque()
        fin = None

        def flush_pv_all():
            while pending_pv:
                emit_pv(*pending_pv.popleft()[1])

        def flush_pv_due(git):
            while pending_pv and pending_pv[0][0] <= git:
                emit_pv(*pending_pv.popleft()[1])
        fin2_done = None  # 2-chunk fin group ready for tr/out

        for pi in range(len(PAIRS)):
            for c in range(NCH):
                ci = pi * NCH + c
                st = st_cur
                out_ps = ps_out.tile([VW, 2 * CH], DT.float32, tag="out", name="out")
                qsl = slice(c * 4, c * 4 + 4)
                for t in range(NT):
                    if t == 0 and fin is not None:
                        flush_pv_all()
                        fin2 = emit_fin_copy(fin)
                        if fin["c"] % 2 == 1:
                            fin2_done = fin2
                        else:
                            st["fin2_open"] = fin2
                        fin = None
                    scp = ps_sc.tile([P, 2 * CH], DT.float32, tag="sc", name="sc")
                    nc.tensor.matmul(
                        scp[:, :CH],
                        st["kT"][0:64, t, :],
                        st["qT"][0:64, qsl, :],
                        start=True, stop=True,
                        tile_position=(0, 0),
                    )
                    nc.tensor.matmul(
                        scp[:, CH:],
                        st["kT"][64:128, t, :],
                        st["qT"][64:128, qsl, :],
                        start=True, stop=True,
                        tile_position=(64, 0),
                    )
                    probs = pr.tile([P, 2 * CH], DT.bfloat16, tag="probs", name="probs")
                    if t in SCALAR_T:
                        nc.scalar.activation(probs[:], scp[:], AF.Exp)
                    else:
                        nc.vector.tensor_scalar(
                            probs[:].bitcast(DT.int16),
                            scp[:], A16, B16, OP.mult, OP.add,
                        )
                    git = ci * NT + t
                    flush_pv_due(git)
                    if t == 0:
                        delay = PVD_T0
                    else:
                        delay = PVD_DVE if t not in SCALAR_T else PVD_SCALAR
                    # FIFO pop: a later-t entry never overtakes an earlier one,
                    # so effective delay is max over earlier entries; keep
                    # per-t delays monotone-compatible.
                    pending_pv.append((git + delay, (st, out_ps, probs, t)))

                    if t == 1 and fin2_done is not None:
                        emit_fin_tr(fin2_done)
                    if t == 3 and fin2_done is not None:
                        emit_fin_out(fin2_done)
                        fin2_done = None
                    # stream remaining setup: ~3 ops per even iteration
                    if pending_setup and t % 2 == 0:
                        for _ in range(3):
                            if pending_setup:
                                pending_setup.pop(0)()
                    # kick off next pair's setup in the 2nd chunk of this pair
                    if c == 1 and t == 0 and pi + 1 < len(PAIRS):
                        st_next = alloc_pair(pi + 1)
                        pending_setup = p1_stream_ops(st_next)

                fin = {"out_ps": out_ps, "pi": pi, "c": c}
                if c % 2 == 1:
                    fin["fin2"] = st.pop("fin2_open")
                if ci == len(PAIRS) * NCH - 1:
                    flush_pv_all()
                    fin2 = emit_fin_copy(fin)
                    fin = None
                    # tail: per-head tr -> recip/mult -> store pipelining
                    hA, _ = PAIRS[fin2["pi"]]
                    c0 = fin2["c0"]
                    finT = sb.tile([P, 2, 8, VW], DT.bfloat16, tag="finT",
                                   name="finT")
                    for hi in (0, 1):
                        nc.sync.dma_start_transpose(
                            finT[:, hi], fin2["outT"][:, hi]
                        )
                        recip = sb.tile([P, 8, 1], DT.float32,
                                        tag=f"recipT{hi}", name="recipT")
                        nc.vector.reciprocal(recip[:], finT[:, hi, :, D : D + 1])
                        outn = sb.tile([P, 8, D], DT.float32,
                                       tag=f"outnT{hi}", name="outnT")
                        nc.vector.tensor_tensor(
                            outn[:],
                            finT[:, hi, :, :D],
                            recip[:].to_broadcast((P, 8, D)),
                            OP.mult,
                        )
                        nc.sync.dma_start(
                            out_ext[hA + hi].rearrange(RLOAD, p=P, b=2)[
                                :, c0 * 2 : c0 * 2 + 4, :, :
                            ],
                            outn[:].rearrange("p (a b) d -> p a b d", b=2),
                        )

            st_cur = st_next
            st_next = None

    nc.compile()
    return nc


_NC = None


def _get_nc():
    global _NC
    if _NC is None:
        _NC = build()
    return _NC


def kernel(q: np.ndarray, k: np.ndarray, v: np.ndarray) -> np.ndarray:
    qf = np.ascontiguousarray(q, dtype=np.float32).reshape(B * H, S, D)
    kf = np.ascontiguousarray(k, dtype=np.float32).reshape(B * H, S, D)
    vf = np.ascontiguousarray(v, dtype=np.float32).reshape(B * H, S, D)
    in_maps = [
        {
            "q": qf[c * HPC : (c + 1) * HPC],
            "k": kf[c * HPC : (c + 1) * HPC],
            "v": vf[c * HPC : (c + 1) * HPC],
        }
        for c in range(NCORES)
    ]
    nc = _get_nc()
    res = run_bass_kernel_spmd(nc, in_maps, core_ids=list(range(NCORES)))
    out = np.concatenate([res.results[c]["out"] for c in range(NCORES)], axis=0)
    return out.reshape(B, H, S, D)


# revision 7
# speedup vs baseline: 1.0643x; 1.0643x over previous
"""Trainium2 Bass kernel for nn_AttentionCore64: softmax(Q@K^T)@V (raw exp,
no scaling), B=2 H=16 S=2048 D=64, f32 in/out. B*H sharded over 8 cores.

Final: v1 steady-state structure (per-2-chunk fin on SP, 11/5 ACT/DVE
exp split, PV delays 2/4) with boundary-stall fixes:
- chunk's FIRST PV delayed to t+5 + strict t-order PV FIFO (start=True
  must stay first per accumulator): the out_ps WAR on the previous
  chunk's fin copies no longer head-blocks the in-order PE queue;
- both fin staging copies on DVE so the ACT exp stream is gapless
  across boundaries (the last chunk splits them ACT/DVE in parallel
  since ACT is idle by then);
- bootstrap g0 converts on DVE (no Q7 launch in the first-QK chain).
Plus:
- act-table warmup exp at t=0 (overlaps the 1.3us LoadActFuncSet).
- 512B DMA descriptors: interleaved row-pair HBM layout for q/k/v/out
  ("(a p b) d -> p a b d", row s = 256a+2p+b) -> 2x DMA throughput.
  Key order inside each kT tile is permuted, but vx rows (from v in the
  same layout) match; query order permutes score columns, and the store
  uses the same layout, so results are identical.
- Fast bootstrap: k g0-1 + q g0 loads and their transposes first (first
  QK at ~6us, was 13.9us); q g1-3 loads also in the bootstrap so the
  chunk-1 qT transpose has ~10us of lead (was the 1.5-4.8us stall).
- q/k fp32->bf16 converts on GPSIMD (was ACT/DVE): the convert chain no
  longer queues behind exp bursts.
- pair-1 setup kicked off at chunk 1 (was chunk 2): +13.6us of lead.
"""

import numpy as np
from contextlib import ExitStack

import concourse.tile as tile
import concourse.mybir as mybir
from concourse import bacc
from concourse.bass_utils import run_bass_kernel_spmd

B, H, S, D = 2, 16, 2048, 64
NCORES = 8
HPC = (B * H) // NCORES  # 4 heads per core

P = 128
CH = 512            # queries per chunk
NCH = S // CH       # 4 chunks per pair
NT = S // P         # 16 key tiles
NO = S // P         # 16 query o-tiles
DT = mybir.dt
AF = mybir.ActivationFunctionType
OP = mybir.AluOpType

PAIRS = [(0, 1), (2, 3)]
VW = 80             # padded PV weight cols: 64 v + 1 ones + 15 zeros

A16 = float(128.0 / np.log(2.0))
B16 = 16256.0 - 7.30  # calibrated: E[schraudolph/exp] = 1.0003

RLOAD = "(a p b) d -> p a b d"  # interleaved row-pair layout (512B desc)

DVE_T = frozenset((2, 5, 8, 11, 14))
SCALAR_T = frozenset(t for t in range(16) if t not in DVE_T)
PVD_SCALAR = 2
PVD_DVE = 4
PVD_T0 = 5


def build(reps=None):
    nc = bacc.Bacc("TRN2", target_bir_lowering=False, debug=False)
    q_ext = nc.dram_tensor("q", [HPC, S, D], DT.float32, kind="ExternalInput").ap()
    k_ext = nc.dram_tensor("k", [HPC, S, D], DT.float32, kind="ExternalInput").ap()
    v_ext = nc.dram_tensor("v", [HPC, S, D], DT.float32, kind="ExternalInput").ap()
    out_ext = nc.dram_tensor("out", [HPC, S, D], DT.float32, kind="ExternalOutput").ap()

    with tile.TileContext(nc) as tc, ExitStack() as ctx:
        if reps is not None:
            ctx.enter_context(tc.For_i(0, reps))
        sb = ctx.enter_context(tc.tile_pool(name="sb", bufs=2))
        pr = ctx.enter_context(tc.tile_pool(name="pr", bufs=6))
        ps_sc = ctx.enter_context(tc.tile_pool(name="ps_sc", bufs=3, space="PSUM"))
        ps_out = ctx.enter_context(tc.tile_pool(name="ps_out", bufs=1, space="PSUM"))

        # act-table warmup: first Exp triggers the 1.3us LoadActFuncSet.
        warm = sb.tile([1, 8], DT.float32, tag="warm", name="warm")
        warm_o = sb.tile([1, 8], DT.float32, tag="warm_o", name="warm_o")
        nc.gpsimd.memset(warm[:], 0.0)
        nc.scalar.activation(warm_o[:], warm[:], AF.Exp)

        # ---------------- per-pair state ----------------
        def alloc_pair(pi):
            st = {"pi": pi}
            for nm in ("q_nat", "k_nat", "v_nat"):
                st[nm] = sb.tile([P, 2, NO, D], DT.float32, tag=nm, name=nm)
            for nm in ("qbf", "kbf"):
                st[nm] = sb.tile([P, NO, 2, D], DT.bfloat16, tag=nm, name=nm)
            for nm in ("qT", "kT"):
                st[nm] = sb.tile([P, NO, P], DT.bfloat16, tag=nm, name=nm)
            st["vx"] = sb.tile([P, NT, 2, VW], DT.bfloat16, tag="vx", name="vx")
            return st

        def load_head(st, which, hi, g0, gn):
            """One 3D DMA: o-tile groups [g0, g0+gn) of one head (512B desc)."""
            hA, _ = PAIRS[st["pi"]]
            ext = {"q": q_ext, "k": k_ext, "v": v_ext}[which]
            dst = st[{"q": "q_nat", "k": "k_nat", "v": "v_nat"}[which]]
            sl = slice(g0 * 2, (g0 + gn) * 2)
            nc.sync.dma_start(
                dst[:, hi].rearrange("p (a b) d -> p a b d", b=2)[:, sl, :, :],
                ext[hA + hi].rearrange(RLOAD, p=P, b=2)[:, sl, :, :],
            )

        def convert_groups(st, which, g0, gn, eng=None):
            """fp32 head-major -> bf16 o-major for groups [g0, g0+gn), GPSIMD.
            Per-head copies: Q7 handles the 3D strided AP (4D crashes it)."""
            src = st[{"q": "q_nat", "k": "k_nat"}[which]]
            dst = st[{"q": "qbf", "k": "kbf"}[which]]
            sl = slice(g0 * 4, (g0 + gn) * 4)
            if eng == "dve":
                nc.vector.tensor_copy(
                    dst[:, sl, :, :],
                    src[:, :, sl, :].rearrange("p h o d -> p o h d"),
                )
                return
            for hi in (0, 1):
                nc.gpsimd.tensor_copy(dst[:, sl, hi, :], src[:, hi, sl, :])

        def vx_groups(st, g0, gn):
            """Build vx = [v | 1 | zeros] bf16 for groups [g0, g0+gn) (GPSIMD)."""
            sl = slice(g0 * 4, (g0 + gn) * 4)
            for hi in (0, 1):
                nc.gpsimd.tensor_copy(
                    st["vx"][:, sl, hi, :D], st["v_nat"][:, hi, sl, :]
                )
            nc.gpsimd.memset(st["vx"][:, sl, :, D], 1.0)
            nc.gpsimd.memset(st["vx"][:, sl, :, D + 1 :], 0.0)

        def tr_groups(st, which, g0, gn, eng=None):
            """One XBAR DMA transposing o-tiles [g0*4, (g0+gn)*4) blockwise."""
            src = st[{"q": "qbf", "k": "kbf"}[which]]
            dst = st[{"q": "qT", "k": "kT"}[which]]
            sl = slice(g0 * 4, (g0 + gn) * 4)
            e = nc.scalar if eng == "act" else nc.sync
            e.dma_start_transpose(dst[:, sl, :], src[:, sl, :, :])

        # ---------------- finalization (per 2-chunk group, on SP) ----------
        def emit_fin_copy(fin):
            out_ps = fin["out_ps"]
            cc = fin["c"] % 2
            if cc == 0:
                fin2 = {"pi": fin["pi"], "c0": fin["c"]}
                fin2["outT"] = sb.tile(
                    [VW, 2, 2, CH], DT.bfloat16, tag="outT", name="outT"
                )
            else:
                fin2 = fin["fin2"]
            outT = fin2["outT"]
            if fin.get("last"):
                nc.scalar.copy(outT[:, 0, cc, :], out_ps[:, :CH])
            else:
                nc.vector.tensor_copy(outT[:, 0, cc, :], out_ps[:, :CH])
            nc.vector.tensor_copy(outT[:, 1, cc, :], out_ps[:, CH:])
            return fin2

        def emit_fin_tr(fin2):
            finT = sb.tile([P, 2, 8, VW], DT.bfloat16, tag="finT", name="finT")
            nc.sync.dma_start_transpose(finT[:], fin2["outT"][:])
            fin2["finT"] = finT

        def emit_fin_out(fin2):
            finT = fin2["finT"]  # [128, hi, (cc j), 80]
            hA, _ = PAIRS[fin2["pi"]]
            c0 = fin2["c0"]
            recip = sb.tile([P, 2, 8, 1], DT.float32, tag="recip", name="recip")
            nc.vector.reciprocal(recip[:], finT[:, :, :, D : D + 1])
            outn = sb.tile([P, 2, 8, D], DT.float32, tag="outn", name="outn")
            nc.vector.tensor_tensor(
                outn[:],
                finT[:, :, :, :D],
                recip[:].to_broadcast((P, 2, 8, D)),
                OP.mult,
            )
            for hi in (0, 1):
                nc.sync.dma_start(
                    out_ext[hA + hi].rearrange(RLOAD, p=P, b=2)[
                        :, c0 * 2 : c0 * 2 + 4, :, :
                    ],
                    outn[:, hi].rearrange("p (a b) d -> p a b d", b=2),
                )

        def emit_pv(st, out_ps, probs, t):
            nc.tensor.matmul(
                out_ps[:, :CH], st["vx"][:, t, 0, :], probs[:, :CH],
                start=(t == 0), stop=(t == NT - 1),
            )
            nc.tensor.matmul(
                out_ps[:, CH:], st["vx"][:, t, 1, :], probs[:, CH:],
                start=(t == 0), stop=(t == NT - 1),
            )

        # ---------------- pair-setup emission scheduling ----------------
        def p1_stream_ops(st):
            # plain loads strictly before the transposes that wait on their
            # converts (a waiting DMA head-blocks the SP sequencer); Pool
            # order = conv k -> vx -> conv q (by downstream deadline).
            return [
                lambda: load_head(st, "k", 0, 0, 4),
                lambda: load_head(st, "k", 1, 0, 4),
                lambda: load_head(st, "q", 0, 0, 4),
                lambda: load_head(st, "q", 1, 0, 4),
                lambda: load_head(st, "v", 0, 0, 4),
                lambda: load_head(st, "v", 1, 0, 4),
                lambda: convert_groups(st, "k", 0, 2),
                lambda: convert_groups(st, "k", 2, 2),
                lambda: tr_groups(st, "k", 0, 4),
                lambda: vx_groups(st, 0, 2),
                lambda: vx_groups(st, 2, 2),
                lambda: convert_groups(st, "q", 0, 2),
                lambda: convert_groups(st, "q", 2, 2),
                lambda: tr_groups(st, "q", 0, 4),
            ]

        # ------- bootstrap: entire pair-0 setup, minimal QK(0,0) chain first.
        # SP: the k g0-1 / q g0 chains feeding QK(0,0), then all remaining
        # loads, then the transposes that wait on Pool converts.
        st_cur = alloc_pair(0)
        load_head(st_cur, "k", 0, 0, 2)
        load_head(st_cur, "k", 1, 0, 2)
        load_head(st_cur, "q", 0, 0, 1)
        load_head(st_cur, "q", 1, 0, 1)
        convert_groups(st_cur, "k", 0, 1, eng="dve")
        convert_groups(st_cur, "q", 0, 1, eng="dve")
        tr_groups(st_cur, "k", 0, 1, eng="act")
        tr_groups(st_cur, "q", 0, 1, eng="act")
        load_head(st_cur, "v", 0, 0, 2)
        load_head(st_cur, "v", 1, 0, 2)
        load_head(st_cur, "k", 0, 2, 2)
        load_head(st_cur, "k", 1, 2, 2)
        load_head(st_cur, "q", 0, 1, 3)
        load_head(st_cur, "q", 1, 1, 3)
        load_head(st_cur, "v", 0, 2, 2)
        load_head(st_cur, "v", 1, 2, 2)
        convert_groups(st_cur, "k", 1, 1)
        tr_groups(st_cur, "k", 1, 1)
        vx_groups(st_cur, 0, 1)
        vx_groups(st_cur, 1, 1)
        convert_groups(st_cur, "k", 2, 2)
        tr_groups(st_cur, "k", 2, 2)
        vx_groups(st_cur, 2, 1)
        vx_groups(st_cur, 3, 1)
        convert_groups(st_cur, "q", 1, 3)
        tr_groups(st_cur, "q", 1, 3)
        pending_setup = []

        from collections import deque

        st_next = None
        # FIFO popped strictly in t-order: PV(t=0) carries start=True, and
        # with PVD_T0 > later delays a due-sorted flush would emit PV(1)
        # first and the start would zero its contribution.
        pending_pv = deque()
        fin = None

        def flush_pv_all():
            while pending_pv:
                emit_pv(*pending_pv.popleft()[1])

        def flush_pv_due(git):
            while pending_pv and pending_pv[0][0] <= git:
                emit_pv(*pending_pv.popleft()[1])
        fin2_done = None  # 2-chunk fin group ready for tr/out

        for pi in range(len(PAIRS)):
            for c in range(NCH):
                ci = pi * NCH + c
                st = st_cur
                out_ps = ps_out.tile([VW, 2 * CH], DT.float32, tag="out", name="out")
                qsl = slice(c * 4, c * 4 + 4)
                for t in range(NT):
                    if t == 0 and fin is not None:
                        flush_pv_all()
                        fin2 = emit_fin_copy(fin)
                        if fin["c"] % 2 == 1:
                            fin2_done = fin2
                        else:
                            st["fin2_open"] = fin2
                        fin = None
                    scp = ps_sc.tile([P, 2 * CH], DT.float32, tag="sc", name="sc")
                    nc.tensor.matmul(
                        scp[:, :CH],
                        st["kT"][0:64, t, :],
                        st["qT"][0:64, qsl, :],
                        start=True, stop=True,
                        tile_position=(0, 0),
                    )
                    nc.tensor.matmul(
                        scp[:, CH:],
                        st["kT"][64:128, t, :],
                        st["qT"][64:128, qsl, :],
                        start=True, stop=True,
                        tile_position=(64, 0),
                    )
                    probs = pr.tile([P, 2 * CH], DT.bfloat16, tag="probs", name="probs")
                    if t in SCALAR_T:
                        nc.scalar.activation(probs[:], scp[:], AF.Exp)
                    else:
                        nc.vector.tensor_scalar(
                            probs[:].bitcast(DT.int16),
                            scp[:], A16, B16, OP.mult, OP.add,
                        )
                    git = ci * NT + t
                    flush_pv_due(git)
                    if t == 0:
                        delay = PVD_T0
                    else:
                        delay = PVD_DVE if t not in SCALAR_T else PVD_SCALAR
                    # FIFO pop: a later-t entry never overtakes an earlier one,
                    # so effective delay is max over earlier entries; keep
                    # per-t delays monotone-compatible.
                    pending_pv.append((git + delay, (st, out_ps, probs, t)))

                    if t == 1 and fin2_done is not None:
                        emit_fin_tr(fin2_done)
                    if t == 3 and fin2_done is not None:
                        emit_fin_out(fin2_done)
                        fin2_done = None
                    # stream remaining setup: ~3 ops per even iteration
                    if pending_setup and t % 2 == 0:
                        for _ in range(3):
                            if pending_setup:
                                pending_setup.pop(0)()
                    # kick off next pair's setup in the 2nd chunk of this pair
                    if c == 1 and t == 0 and pi + 1 < len(PAIRS):
                        st_next = alloc_pair(pi + 1)
                        pending_setup = p1_stream_ops(st_next)

                fin = {"out_ps": out_ps, "pi": pi, "c": c}
                if c % 2 == 1:
                    fin["fin2"] = st.pop("fin2_open")
                if ci == len(PAIRS) * NCH - 1:
                    flush_pv_all()
                    fin["last"] = True
                    fin2 = emit_fin_copy(fin)
                    fin = None
                    # tail: per-head tr -> recip/mult -> store pipelining
                    hA, _ = PAIRS[fin2["pi"]]
                    c0 = fin2["c0"]
                    finT = sb.tile([P, 2, 8, VW], DT.bfloat16, tag="finT",
                                   name="finT")
                    for hi in (0, 1):
                        nc.sync.dma_start_transpose(
                            finT[:, hi], fin2["outT"][:, hi]
                        )
                        recip = sb.tile([P, 8, 1], DT.float32,
                                        tag=f"recipT{hi}", name="recipT")
                        nc.vector.reciprocal(recip[:], finT[:, hi, :, D : D + 1])
                        outn = sb.tile([P, 8, D], DT.float32,
                                       tag=f"outnT{hi}", name="outnT")
                        nc.vector.tensor_tensor(
                            outn[:],
                            finT[:, hi, :, :D],
                            recip[:].to_broadcast((P, 8, D)),
                            OP.mult,
                        )
                        nc.sync.dma_start(
                            out_ext[hA + hi].rearrange(RLOAD, p=P, b=2)[
                                :, c0 * 2 : c0 * 2 + 4, :, :
                            ],
                            outn[:].rearrange("p (a b) d -> p a b d", b=2),
                        )

            st_cur = st_next
            st_next = None

    nc.compile()
    return nc


_NC = None


def _get_nc():
    global _NC
    if _NC is None:
        _NC = build()
    return _NC


def kernel(q: np.ndarray, k: np.ndarray, v: np.ndarray) -> np.ndarray:
    qf = np.ascontiguousarray(q, dtype=np.float32).reshape(B * H, S, D)
    kf = np.ascontiguousarray(k, dtype=np.float32).reshape(B * H, S, D)
    vf = np.ascontiguousarray(v, dtype=np.float32).reshape(B * H, S, D)
    in_maps = [
        {
            "q": qf[c * HPC : (c + 1) * HPC],
            "k": kf[c * HPC : (c + 1) * HPC],
            "v": vf[c * HPC : (c + 1) * HPC],
        }
        for c in range(NCORES)
    ]
    nc = _get_nc()
    res = run_bass_kernel_spmd(nc, in_maps, core_ids=list(range(NCORES)))
    out = np.concatenate([res.results[c]["out"] for c in range(NCORES)], axis=0)
    return out.reshape(B, H, S, D)
